# revision 1
# baseline (speedup 1.0000x reference)
"""CrossModalAttention kernel for 8x TRN2 NeuronCores (batch data-parallel).

Reference computation (per batch element b, context input is unused):
    qkv = x @ qkv_w + qkv_b            # [N, 3C]
    q, k, v = split(qkv)               # heads H=12, d=64
    attn = softmax(q*scale @ k^T)      # per head, N=1024
    out = (attn @ v) @ proj_w + proj_b # [N, C]

Strategy per core (one batch element each):
  - Host pre-transposes x -> xT [C, N], splits qkv_w into wqk (q part
    pre-scaled by d^-0.5) and wv.
  - Stage 1: qkT [2*C, N] = wqk^T-as-lhsT matmuls (transposed layout, so
    per-head qT/kT slices are directly usable as scores operands), and
    v in natural layout [N, C] stored augmented with a ones column per
    head (v_aug [N, H*65]) so the attn@v matmul also produces softmax
    denominators for free.
  - Stage 2: per head pair (2 heads packed in the 128-partition K dim of
    the PE array): scoresT[k, q] = kT.T-as-lhsT @ qT, exp on ACT
    (scores are O(1); max-subtraction unnecessary), attn@v accumulated
    over k chunks into psum [65, N]: rows 0-63 = out^T, row 64 = sums.
    1/sums via fast Newton reciprocal, broadcast across partitions via
    DMA, normalize with DVE.
  - Stage 3: proj: final[t, :] = outT-as-lhsT @ wproj (+ bias), natural
    layout, DMA out.
  All matmul operands are float32r (TF32-like: ~1.6e-4 rel err, 4x the
  throughput of float32 on the PE).
"""
import numpy as np

import concourse.bass as bass
import concourse.tile as tile
from concourse import bacc, mybir
from concourse.bass_utils import run_bass_kernel_spmd

DIM = 768
NUM_HEADS = 12
HEAD_DIM = 64
B, N = 8, 1024
P = 128
KC = DIM // P          # 6 contraction chunks of 128 over channels
TC = N // P            # 8 token chunks of 128
QC = N // 512          # 2 free-dim chunks of 512 over tokens
HP = NUM_HEADS // 2    # 6 head pairs
VAUG = 65              # v columns per head: 64 v dims + 1 ones column

F32 = mybir.dt.float32
F32R = mybir.dt.float32r
BF16 = mybir.dt.bfloat16
FP16 = mybir.dt.float16


def build_nc(with_qkv_bias: bool, with_proj_bias: bool):
    nc = bacc.Bacc("TRN2", target_bir_lowering=False, debug=False)

    xT_d = nc.dram_tensor("xT", [DIM, N], F32, kind="ExternalInput")
    wqk_d = nc.dram_tensor("wqk", [DIM, 2 * DIM], F32, kind="ExternalInput")
    wv_d = nc.dram_tensor("wv", [DIM, DIM], F32, kind="ExternalInput")
    wproj_d = nc.dram_tensor("wproj", [DIM, DIM], F32, kind="ExternalInput")
    bqk_d = nc.dram_tensor("bqk", [1, 2 * DIM], F32, kind="ExternalInput")
    bv_d = nc.dram_tensor("bv", [1, DIM], F32, kind="ExternalInput")
    bproj_d = nc.dram_tensor("bproj", [1, DIM], F32, kind="ExternalInput")
    out_d = nc.dram_tensor("out", [N, DIM], F32, kind="ExternalOutput")

    with tile.TileContext(nc) as tc:
        with (
            tc.tile_pool(name="qk_sb", bufs=1) as qk_pool,
            tc.tile_pool(name="vaug_sb", bufs=1) as vaug_pool,
            tc.tile_pool(name="consts", bufs=1) as consts,
            tc.tile_pool(name="expT", bufs=4) as exp_pool,
        ):
            # ---- constants ----
            if with_qkv_bias or with_proj_bias:
                ones_f32 = consts.tile([1, N], F32)
                nc.vector.memset(ones_f32[:], 1.0)
                ones_row = consts.tile([1, N], F32R)
                nc.vector.tensor_copy(ones_row[:], ones_f32[:])

            if with_qkv_bias:
                bqk_sb = consts.tile([1, 2 * DIM], F32R)
                nc.sync.dma_start(out=bqk_sb[:], in_=bqk_d[:].bitcast(F32R))
                bv_sb = consts.tile([1, DIM], F32R)
                nc.sync.dma_start(out=bv_sb[:], in_=bv_d[:].bitcast(F32R))
            if with_proj_bias:
                bproj_sb = consts.tile([1, DIM], F32R)
                nc.sync.dma_start(out=bproj_sb[:], in_=bproj_d[:].bitcast(F32R))

            # ---- persistent tiles ----
            qkT = [
                qk_pool.tile([P, N], F32R, name=f"qkT{m}") for m in range(2 * KC)
            ]  # m-chunk m covers qk rows m*128..m*128+127 (q first, then k)
            v_aug = [
                vaug_pool.tile([P, NUM_HEADS * VAUG], FP16, name=f"vaug{t}")
                for t in range(TC)
            ]

            # ================= stage 1 =================
            with (
                tc.tile_pool(name="xT_sb", bufs=1) as xT_pool,
                tc.tile_pool(name="w_sb", bufs=1) as w_pool,
                tc.tile_pool(name="ps1", bufs=3, space="PSUM") as ps1,
            ):
                xT = [xT_pool.tile([P, N], F32R, name=f"xT{c}") for c in range(KC)]
                wqk = [
                    w_pool.tile([P, 2 * DIM], F32R, name=f"wqk{c}") for c in range(KC)
                ]
                wv = [w_pool.tile([P, DIM], F32R, name=f"wv{c}") for c in range(KC)]
                for c in range(KC):
                    sl = slice(c * P, (c + 1) * P)
                    nc.sync.dma_start(out=xT[c][:], in_=xT_d[sl, :].bitcast(F32R))
                    nc.sync.dma_start(out=wqk[c][:], in_=wqk_d[sl, :].bitcast(F32R))
                    nc.sync.dma_start(out=wv[c][:], in_=wv_d[sl, :].bitcast(F32R))

                # qkT[m] = sum_c wqk[c][:, m-slice].T @ xT[c]   (+ bias ⊗ ones)
                for m in range(2 * KC):
                    ps = ps1.tile([P, N], F32, name=f"ps_qk{m}", tag="ps1")
                    msl = slice(m * P, (m + 1) * P)
                    for c in range(KC):
                        for q in range(QC):
                            qsl = slice(q * 512, (q + 1) * 512)
                            nc.tensor.matmul(
                                ps[:, qsl],
                                wqk[c][:, msl],
                                xT[c][:, qsl],
                                start=(c == 0),
                                stop=(c == KC - 1) and not with_qkv_bias,
                            )
                    if with_qkv_bias:
                        for q in range(QC):
                            qsl = slice(q * 512, (q + 1) * 512)
                            nc.tensor.matmul(
                                ps[:, qsl],
                                bqk_sb[:, msl],
                                ones_row[:, qsl],
                                start=False,
                                stop=True,
                            )
                    nc.vector.tensor_copy(qkT[m][:], ps[:])

                # v natural: v[t] = sum_c xT[c][:, t-slice].T @ wv[c] (+ ones ⊗ bias)
                for t in range(TC):
                    ps = ps1.tile([P, DIM], F32, name=f"ps_v{t}", tag="ps1")
                    tsl = slice(t * P, (t + 1) * P)
                    for c in range(KC):
                        for nsl in (slice(0, 512), slice(512, DIM)):
                            nc.tensor.matmul(
                                ps[:, nsl],
                                xT[c][:, tsl],
                                wv[c][:, nsl],
                                start=(c == 0),
                                stop=(c == KC - 1) and not with_qkv_bias,
                            )
                    if with_qkv_bias:
                        for nsl in (slice(0, 512), slice(512, DIM)):
                            nc.tensor.matmul(
                                ps[:, nsl],
                                ones_row[:, t * P : t * P + P],
                                bv_sb[:, nsl],
                                start=False,
                                stop=True,
                            )
                    va = v_aug[t]
                    va3 = va[:].rearrange("p (h e) -> p h e", e=VAUG)
                    nc.vector.memset(va3[:, :, 64:65], 1.0)
                    nc.vector.tensor_copy(
                        va3[:, :, 0:64],
                        ps[:].rearrange("p (h d) -> p h d", d=HEAD_DIM),
                    )

            # ================= stage 2 + 3 =================
            with (
                tc.tile_pool(name="s23", bufs=1) as s23,
                tc.tile_pool(name="rep", bufs=3) as rep_pool,
                tc.tile_pool(name="fin", bufs=3) as fin_pool,
            ):
                outT = [
                    s23.tile([P, N], F32R, name=f"outT{p}") for p in range(HP)
                ]  # row block p = heads 2p (parts 0-63), 2p+1 (parts 64-127)
                wproj = [
                    s23.tile([P, DIM], F32R, name=f"wproj{c}") for c in range(KC)
                ]
                for c in range(KC):
                    nc.sync.dma_start(
                        out=wproj[c][:],
                        in_=wproj_d[c * P : (c + 1) * P, :].bitcast(F32R),
                    )
                with (
                    tc.tile_pool(name="norm", bufs=2) as norm_pool,
                    tc.tile_pool(name="dramp", bufs=1, space="DRAM") as dram_pool,
                    tc.tile_pool(name="ps_sc", bufs=2, space="PSUM") as ps_sc,
                    tc.tile_pool(name="ps_av", bufs=2, space="PSUM") as ps_av,
                ):
                    recip_d = dram_pool.tile([NUM_HEADS, N], F32)
                    for p in range(HP):
                        qT = qkT[p]
                        kT = qkT[KC + p]
                        av_A = ps_av.tile([VAUG, N], F32, name=f"av{p}A", tag="av")
                        av_B = ps_av.tile([VAUG, N], F32, name=f"av{p}B", tag="av")
                        for kc in range(TC):
                            ksl = slice(kc * P, (kc + 1) * P)
                            sc_A = ps_sc.tile([P, N], F32, name=f"sc{p}_{kc}A", tag="sc")
                            sc_B = ps_sc.tile([P, N], F32, name=f"sc{p}_{kc}B", tag="sc")
                            for q in range(QC):
                                qsl = slice(q * 512, (q + 1) * 512)
                                nc.tensor.matmul(
                                    sc_A[:, qsl], kT[0:64, ksl], qT[0:64, qsl],
                                    start=True, stop=True,
                                )
                                nc.tensor.matmul(
                                    sc_B[:, qsl], kT[64:128, ksl], qT[64:128, qsl],
                                    start=True, stop=True,
                                )
                            eT_A = exp_pool.tile([P, N], FP16, name=f"e{p}_{kc}A", tag="e")
                            eT_B = exp_pool.tile([P, N], FP16, name=f"e{p}_{kc}B", tag="e")
                            nc.scalar.activation(
                                eT_A[:], sc_A[:], mybir.ActivationFunctionType.Exp
                            )
                            nc.scalar.activation(
                                eT_B[:], sc_B[:], mybir.ActivationFunctionType.Exp
                            )
                            for q in range(QC):
                                qsl = slice(q * 512, (q + 1) * 512)
                                nc.tensor.matmul(
                                    av_A[:, qsl],
                                    v_aug[kc][:, 2 * p * VAUG : (2 * p + 1) * VAUG],
                                    eT_A[:, qsl],
                                    start=(kc == 0), stop=(kc == TC - 1),
                                )
                                nc.tensor.matmul(
                                    av_B[:, qsl],
                                    v_aug[kc][:, (2 * p + 1) * VAUG : (2 * p + 2) * VAUG],
                                    eT_B[:, qsl],
                                    start=(kc == 0), stop=(kc == TC - 1),
                                )
                        # per-pair normalize: sums -> 1/sums -> broadcast -> mul
                        for hh, av in ((2 * p, av_A), (2 * p + 1, av_B)):
                            sums_t = norm_pool.tile(
                                [1, N], F32, name=f"sums{hh}", tag="sums"
                            )
                            recip_t = norm_pool.tile(
                                [1, N], F32, name=f"recip{hh}", tag="recip"
                            )
                            nc.vector.tensor_copy(sums_t[:], av[64:65, :])
                            nc.vector.reciprocal_approx_fast(
                                out=recip_t[:], in_=sums_t[:]
                            )
                            nc.sync.dma_start(
                                out=recip_d[hh : hh + 1, :], in_=recip_t[:]
                            )
                        for hh, av in ((2 * p, av_A), (2 * p + 1, av_B)):
                            rep = rep_pool.tile([64, N], F32, name=f"rep{hh}", tag="rep")
                            nc.sync.dma_start(
                                out=rep[:],
                                in_=recip_d[hh : hh + 1, :].to_broadcast([64, N]),
                            )
                            half = slice(0, 64) if hh % 2 == 0 else slice(64, 128)
                            nc.vector.tensor_tensor(
                                out=outT[p][half, :],
                                in0=av[0:64, :],
                                in1=rep[:],
                                op=mybir.AluOpType.mult,
                            )

                # ---- stage 3: proj ----
                with tc.tile_pool(name="ps_pj", bufs=2, space="PSUM") as ps_pj:
                    for t in range(TC):
                        ps = ps_pj.tile([P, DIM], F32, name=f"pj{t}", tag="pj")
                        tsl = slice(t * P, (t + 1) * P)
                        for c in range(KC):
                            for nsl in (slice(0, 512), slice(512, DIM)):
                                nc.tensor.matmul(
                                    ps[:, nsl],
                                    outT[c][:, tsl],
                                    wproj[c][:, nsl],
                                    start=(c == 0),
                                    stop=(c == KC - 1) and not with_proj_bias,
                                )
                        if with_proj_bias:
                            for nsl in (slice(0, 512), slice(512, DIM)):
                                nc.tensor.matmul(
                                    ps[:, nsl],
                                    ones_row[:, t * P : t * P + P],
                                    bproj_sb[:, nsl],
                                    start=False,
                                    stop=True,
                                )
                        fin = fin_pool.tile([P, DIM], F32, name=f"fin{t}", tag="fin")
                        nc.vector.tensor_copy(fin[:], ps[:])
                        nc.sync.dma_start(out=out_d[tsl, :], in_=fin[:])

    nc.compile()
    return nc


_NC_CACHE = {}


def kernel(**inputs) -> np.ndarray:
    x = np.asarray(inputs["x"], dtype=np.float32)
    qkv_w = np.asarray(inputs["qkv_w"], dtype=np.float32)
    qkv_b = np.asarray(inputs["qkv_b"], dtype=np.float32)
    proj_w = np.asarray(inputs["proj_w"], dtype=np.float32)
    proj_b = np.asarray(inputs["proj_b"], dtype=np.float32)
    # context is unused by the reference layer.

    scale = HEAD_DIM ** -0.5
    wqk = qkv_w[:, : 2 * DIM].copy()
    wqk[:, :DIM] *= scale
    wv = np.ascontiguousarray(qkv_w[:, 2 * DIM :])
    bqk = qkv_b[: 2 * DIM].copy()
    bqk[:DIM] *= scale
    bv = qkv_b[2 * DIM :].copy()

    with_qkv_bias = bool(np.any(qkv_b))
    with_proj_bias = bool(np.any(proj_b))

    key = (with_qkv_bias, with_proj_bias)
    if key not in _NC_CACHE:
        _NC_CACHE[key] = build_nc(*key)
    nc = _NC_CACHE[key]

    base = {
        "wqk": wqk,
        "wv": wv,
        "wproj": proj_w,
        "bqk": bqk.reshape(1, -1),
        "bv": bv.reshape(1, -1),
        "bproj": proj_b.reshape(1, -1),
    }
    in_maps = [
        {**base, "xT": np.ascontiguousarray(x[b].T)} for b in range(B)
    ]
    res = run_bass_kernel_spmd(nc, in_maps, list(range(B)))
    out = np.stack([res.results[b]["out"] for b in range(B)], axis=0)
    return out.astype(np.float32)



# revision 12
# speedup vs baseline: 1.0537x; 1.0537x over previous
"""CrossModalAttention kernel for 8x TRN2 NeuronCores (batch data-parallel).

Reference computation (per batch element b, context input is unused):
    qkv = x @ qkv_w + qkv_b            # [N, 3C]
    q, k, v = split(qkv)               # heads H=12, d=64
    attn = softmax(q*scale @ k^T)      # per head, N=1024
    out = (attn @ v) @ proj_w + proj_b # [N, C]

Strategy per core (one batch element each). v2: single flat scope so the
Tile list-scheduler can interleave the qkv/v/proj GEMMs as PE "filler"
between attention iterations — the PE<->ACT exp ping-pong otherwise
leaves the PE idle in ~1-3us gaps, which trips the HAM activity
throttle to half clock (measured: matmuls at ~1.8-2.6 cyc/col instead
of 1). Keeping the PE saturated holds the clock at 2.4GHz.

  - Host pre-transposes x -> xT [C, N], splits qkv_w into wqk (q part
    pre-scaled by d^-0.5) and wv.
  - qkT [2*C, N] in fp16 (transposed layout: per-head qT/kT slices are
    directly usable as scores operands; fp16 halves SBUF and enables
    fast weight load). v natural [N, C] in fp16, augmented with a ones
    column per head (v_aug [N, H*65]) so attn@v also produces softmax
    denominators.
  - Per head pair (2 heads in the 128 partitions): scoresT[k, q] =
    kT-as-lhsT @ qT (K=64 halves), exp on ACT (scores O(1), no max
    subtraction), attn@v accumulated over k chunks into psum [65, N]:
    rows 0-63 = out^T, row 64 = sums. 1/sums via DVE fast reciprocal
    straight from PSUM, partition-broadcast via DRAM round trip,
    normalize with DVE into fp16 outT.
  - proj: final[t, :] = outT-as-lhsT @ wproj, natural layout, DMA out.
  PSUM budget (8 banks): scores ring 2x[128,1024] = 4 (also carries the
  qkv/v/proj filler groups), av A+B 2x[65,1024] = 4.
"""
import numpy as np

import concourse.bass as bass
import concourse.tile as tile
from concourse import bacc, mybir
from concourse.bass_utils import run_bass_kernel_spmd

DIM = 768
NUM_HEADS = 12
HEAD_DIM = 64
B, N = 8, 1024
P = 128
KC = DIM // P          # 6 contraction chunks of 128 over channels
TC = N // P            # 8 token chunks of 128
QC = N // 512          # 2 free-dim chunks of 512 over tokens
HP = NUM_HEADS // 2    # 6 head pairs
VAUG = 65              # v columns per head: 64 v dims + 1 ones column

F32 = mybir.dt.float32
F32R = mybir.dt.float32r
FP16 = mybir.dt.float16


def build_nc(with_qkv_bias: bool, with_proj_bias: bool):
    nc = bacc.Bacc("TRN2", target_bir_lowering=False, debug=False)

    xT_d = nc.dram_tensor("xT", [DIM, N], F32, kind="ExternalInput")
    wqk_d = nc.dram_tensor("wqk", [DIM, 2 * DIM], F32, kind="ExternalInput")
    wv_d = nc.dram_tensor("wv", [DIM, DIM], F32, kind="ExternalInput")
    wproj_d = nc.dram_tensor("wproj", [DIM, DIM], FP16, kind="ExternalInput")
    bqk_d = nc.dram_tensor("bqk", [1, 2 * DIM], F32, kind="ExternalInput")
    bv_d = nc.dram_tensor("bv", [1, DIM], F32, kind="ExternalInput")
    bproj_d = nc.dram_tensor("bproj", [1, DIM], F32, kind="ExternalInput")
    out_d = nc.dram_tensor("out", [N, DIM], F32, kind="ExternalOutput")

    with tile.TileContext(nc) as tc:
        with (
            tc.tile_pool(name="consts", bufs=1) as consts,
            tc.tile_pool(name="inputs", bufs=1) as in_pool,
            tc.tile_pool(name="qk_sb", bufs=1) as qk_pool,
            tc.tile_pool(name="vaug_sb", bufs=1) as vaug_pool,
            tc.tile_pool(name="outT_sb", bufs=1) as outT_pool,
            tc.tile_pool(name="expT", bufs=4) as exp_pool,
            tc.tile_pool(name="norm", bufs=4) as norm_pool,
            tc.tile_pool(name="rep", bufs=3) as rep_pool,
            tc.tile_pool(name="fin", bufs=3) as fin_pool,
            tc.tile_pool(name="dramp", bufs=1, space="DRAM") as dram_pool,
            tc.tile_pool(name="ps_sc", bufs=2, space="PSUM") as ps_sc,
            tc.tile_pool(name="ps_av", bufs=1, space="PSUM") as ps_av,
        ):
            # ---- constants ----
            if with_qkv_bias or with_proj_bias:
                ones_f32 = consts.tile([1, N], F32)
                nc.vector.memset(ones_f32[:], 1.0)
                ones_row = consts.tile([1, N], F32R)
                nc.vector.tensor_copy(ones_row[:], ones_f32[:])
            if with_qkv_bias:
                bqk_sb = consts.tile([1, 2 * DIM], F32R)
                nc.sync.dma_start(out=bqk_sb[:], in_=bqk_d[:].bitcast(F32R))
                bv_sb = consts.tile([1, DIM], F32R)
                nc.sync.dma_start(out=bv_sb[:], in_=bv_d[:].bitcast(F32R))
            if with_proj_bias:
                bproj_sb = consts.tile([1, DIM], F32R)
                nc.sync.dma_start(out=bproj_sb[:], in_=bproj_d[:].bitcast(F32R))

            # ---- input DMAs (xT/wqk interleaved so qkT m=0 can start
            # accumulating as chunks land; wv next, wproj last) ----
            xT = [in_pool.tile([P, N], F32R, name=f"xT{c}") for c in range(KC)]
            wqk = [
                in_pool.tile([P, 2 * DIM], F32R, name=f"wqk{c}") for c in range(KC)
            ]
            wv = [in_pool.tile([P, DIM], F32R, name=f"wv{c}") for c in range(KC)]
            wproj = [
                in_pool.tile([P, DIM], FP16, name=f"wproj{c}") for c in range(KC)
            ]
            for c in range(KC):
                sl = slice(c * P, (c + 1) * P)
                nc.sync.dma_start(out=xT[c][:], in_=xT_d[sl, :].bitcast(F32R))
                nc.sync.dma_start(out=wqk[c][:], in_=wqk_d[sl, :].bitcast(F32R))
            for c in range(KC):
                sl = slice(c * P, (c + 1) * P)
                nc.sync.dma_start(out=wv[c][:], in_=wv_d[sl, :].bitcast(F32R))

            # ---- persistent tiles ----
            qkT = [
                qk_pool.tile([P, N], FP16, name=f"qkT{m}") for m in range(2 * KC)
            ]  # m-chunk m covers qkv channels m*128..m*128+127 (q then k)
            v_aug = [
                vaug_pool.tile([P, NUM_HEADS * VAUG], FP16, name=f"vaug{t}")
                for t in range(TC)
            ]
            outT = [
                outT_pool.tile([P, N], FP16, name=f"outT{p}") for p in range(HP)
            ]  # pair p: head 2p in parts 0-63, head 2p+1 in parts 64-127
            recip_d = dram_pool.tile([NUM_HEADS, N], F32)

            # ---- filler emitters: qkv/v/proj matmul groups the scheduler
            # can slot into PE idle gaps during attention ----
            def emit_qkT(m, pool, tag):
                ps = pool.tile([P, N], F32, name=f"ps_qk{m}", tag=tag)
                msl = slice(m * P, (m + 1) * P)
                for c in range(KC):
                    for q in range(QC):
                        qsl = slice(q * 512, (q + 1) * 512)
                        nc.tensor.matmul(
                            ps[:, qsl],
                            wqk[c][:, msl],
                            xT[c][:, qsl],
                            start=(c == 0),
                            stop=(c == KC - 1) and not with_qkv_bias,
                        )
                if with_qkv_bias:
                    for q in range(QC):
                        qsl = slice(q * 512, (q + 1) * 512)
                        nc.tensor.matmul(
                            ps[:, qsl],
                            bqk_sb[:, msl],
                            ones_row[:, qsl],
                            start=False,
                            stop=True,
                        )
                nc.vector.tensor_copy(qkT[m][:], ps[:])

            def emit_v(t, pool, tag):
                ps = pool.tile([P, DIM], F32, name=f"ps_v{t}", tag=tag)
                tsl = slice(t * P, (t + 1) * P)
                for c in range(KC):
                    for nsl in (slice(0, 512), slice(512, DIM)):
                        nc.tensor.matmul(
                            ps[:, nsl],
                            xT[c][:, tsl],
                            wv[c][:, nsl],
                            start=(c == 0),
                            stop=(c == KC - 1) and not with_qkv_bias,
                        )
                if with_qkv_bias:
                    for nsl in (slice(0, 512), slice(512, DIM)):
                        nc.tensor.matmul(
                            ps[:, nsl],
                            ones_row[:, t * P : t * P + P],
                            bv_sb[:, nsl],
                            start=False,
                            stop=True,
                        )
                va3 = v_aug[t][:].rearrange("p (h e) -> p h e", e=VAUG)
                nc.vector.memset(va3[:, :, 64:65], 1.0)
                nc.vector.tensor_copy(
                    va3[:, :, 0:64],
                    ps[:].rearrange("p (h d) -> p h d", d=HEAD_DIM),
                )

            def emit_proj(t, pool, tag):
                ps = pool.tile([P, DIM], F32, name=f"pj{t}", tag=tag)
                tsl = slice(t * P, (t + 1) * P)
                for c in range(KC):
                    for nsl in (slice(0, 512), slice(512, DIM)):
                        nc.tensor.matmul(
                            ps[:, nsl],
                            outT[c][:, tsl],
                            wproj[c][:, nsl],
                            start=(c == 0),
                            stop=(c == KC - 1) and not with_proj_bias,
                        )
                if with_proj_bias:
                    for nsl in (slice(0, 512), slice(512, DIM)):
                        nc.tensor.matmul(
                            ps[:, nsl],
                            ones_row[:, t * P : t * P + P],
                            bproj_sb[:, nsl],
                            start=False,
                            stop=True,
                        )
                fin = fin_pool.tile([P, DIM], F32, name=f"fin{t}", tag="fin")
                nc.vector.tensor_copy(fin[:], ps[:])
                nc.sync.dma_start(out=out_d[tsl, :], in_=fin[:])

            # Filler queue: (emitter, arg). v chunks first (pair 0 needs
            # v_aug[kc] at its kc-th AV step), then remaining qkT chunks
            # (pair p+1 needs m=p+1, m=7+p before pair p ends).
            fillers = [(emit_v, t) for t in range(2, TC)]
            for m in range(1, KC):
                fillers.append((emit_qkT, m))
                fillers.append((emit_qkT, KC + m))
            fillers.reverse()  # consume from the end

            # ---- prologue: pair 0 operands ----
            emit_qkT(0, ps_sc, "sc")
            emit_qkT(KC, ps_sc, "sc")
            emit_v(0, ps_sc, "sc")
            emit_v(1, ps_sc, "sc")

            # ---- attention: 6 head pairs, filler between iterations ----
            for p in range(HP):
                qT = qkT[p]
                kT = qkT[KC + p]
                av_A = ps_av.tile([VAUG, N], F32, name=f"av{p}A", tag="av")
                av_B = ps_av.tile([VAUG, N], F32, name=f"av{p}B", tag="avB")
                for kc in range(TC):
                    ksl = slice(kc * P, (kc + 1) * P)
                    sc_A = ps_sc.tile([P, N], F32, name=f"sc{p}_{kc}A", tag="sc")
                    sc_B = ps_sc.tile([P, N], F32, name=f"sc{p}_{kc}B", tag="sc")
                    for q in range(QC):
                        qsl = slice(q * 512, (q + 1) * 512)
                        nc.tensor.matmul(
                            sc_A[:, qsl], kT[0:64, ksl], qT[0:64, qsl],
                            start=True, stop=True,
                        )
                        nc.tensor.matmul(
                            sc_B[:, qsl], kT[64:128, ksl], qT[64:128, qsl],
                            start=True, stop=True,
                        )
                    eT_A = exp_pool.tile([P, N], FP16, name=f"e{p}_{kc}A", tag="e")
                    eT_B = exp_pool.tile([P, N], FP16, name=f"e{p}_{kc}B", tag="e")
                    nc.scalar.activation(
                        eT_A[:], sc_A[:], mybir.ActivationFunctionType.Exp
                    )
                    nc.scalar.activation(
                        eT_B[:], sc_B[:], mybir.ActivationFunctionType.Exp
                    )
                    for q in range(QC):
                        qsl = slice(q * 512, (q + 1) * 512)
                        nc.tensor.matmul(
                            av_A[:, qsl],
                            v_aug[kc][:, 2 * p * VAUG : (2 * p + 1) * VAUG],
                            eT_A[:, qsl],
                            start=(kc == 0), stop=(kc == TC - 1),
                        )
                        nc.tensor.matmul(
                            av_B[:, qsl],
                            v_aug[kc][:, (2 * p + 1) * VAUG : (2 * p + 2) * VAUG],
                            eT_B[:, qsl],
                            start=(kc == 0), stop=(kc == TC - 1),
                        )
                    # one filler group after each attention iteration while
                    # any remain (pair 0 consumes v2..v7 + qkT m=1,7; later
                    # pairs 2 qkT groups each)
                    want = 1 if (p > 0 and kc in (2, 5)) or p == 0 else 0
                    for _ in range(want):
                        if fillers:
                            fn, arg = fillers.pop()
                            fn(arg, ps_sc, "sc")
                if p == HP - 2 and not fillers:
                    # wproj can land any time before proj; issue during pair 4
                    for c in range(KC):
                        nc.sync.dma_start(
                            out=wproj[c][:],
                            in_=wproj_d[c * P : (c + 1) * P, :],
                        )
                # ---- normalization: 1/sums from av row 64 (PSUM), DRAM
                # round trip broadcasts across partitions, DVE multiply ----
                for hh, av in ((2 * p, av_A), (2 * p + 1, av_B)):
                    sums_t = norm_pool.tile([1, N], F32, name=f"sums{hh}", tag="sums")
                    recip_t = norm_pool.tile(
                        [1, N], F32, name=f"recip{hh}", tag="recip"
                    )
                    nc.vector.tensor_copy(sums_t[:], av[64:65, :])
                    nc.vector.reciprocal_approx_fast(
                        out=recip_t[:], in_=sums_t[:]
                    )
                    nc.sync.dma_start(out=recip_d[hh : hh + 1, :], in_=recip_t[:])
                for hh, av in ((2 * p, av_A), (2 * p + 1, av_B)):
                    rep = rep_pool.tile([64, N], F32, name=f"rep{hh}", tag="rep")
                    nc.sync.dma_start(
                        out=rep[:],
                        in_=recip_d[hh : hh + 1, :].to_broadcast([64, N]),
                    )
                    half = slice(0, 64) if hh % 2 == 0 else slice(64, 128)
                    nc.vector.tensor_tensor(
                        out=outT[p][half, :],
                        in0=av[0:64, :],
                        in1=rep[:],
                        op=mybir.AluOpType.mult,
                    )

            # ---- proj: sc ring is free once attention is done ----
            for t in range(TC):
                emit_proj(t, ps_sc, "sc")

    nc.compile()
    return nc


_NC_CACHE = {}


def kernel(**inputs) -> np.ndarray:
    x = np.asarray(inputs["x"], dtype=np.float32)
    qkv_w = np.asarray(inputs["qkv_w"], dtype=np.float32)
    qkv_b = np.asarray(inputs["qkv_b"], dtype=np.float32)
    proj_w = np.asarray(inputs["proj_w"], dtype=np.float32)
    proj_b = np.asarray(inputs["proj_b"], dtype=np.float32)
    # context is unused by the reference layer.

    scale = HEAD_DIM ** -0.5
    wqk = qkv_w[:, : 2 * DIM].copy()
    wqk[:, :DIM] *= scale
    wv = np.ascontiguousarray(qkv_w[:, 2 * DIM :])
    bqk = qkv_b[: 2 * DIM].copy()
    bqk[:DIM] *= scale
    bv = qkv_b[2 * DIM :].copy()

    with_qkv_bias = bool(np.any(qkv_b))
    with_proj_bias = bool(np.any(proj_b))

    key = (with_qkv_bias, with_proj_bias)
    if key not in _NC_CACHE:
        _NC_CACHE[key] = build_nc(*key)
    nc = _NC_CACHE[key]

    base = {
        "wqk": wqk,
        "wv": wv,
        "wproj": proj_w.astype(np.float16),
        "bqk": bqk.reshape(1, -1),
        "bv": bv.reshape(1, -1),
        "bproj": proj_b.reshape(1, -1),
    }
    in_maps = [
        {**base, "xT": np.ascontiguousarray(x[b].T)} for b in range(B)
    ]
    res = run_bass_kernel_spmd(nc, in_maps, list(range(B)))
    out = np.stack([res.results[b]["out"] for b in range(B)], axis=0)
    return out.astype(np.float32)


# revision 13
# speedup vs baseline: 1.1903x; 1.1297x over previous
"""CrossModalAttention kernel for 8x TRN2 NeuronCores (batch data-parallel).

Reference computation (per batch element b, context input is unused):
    qkv = x @ qkv_w + qkv_b            # [N, 3C]
    q, k, v = split(qkv)               # heads H=12, d=64
    attn = softmax(q*scale @ k^T)      # per head, N=1024
    out = (attn @ v) @ proj_w + proj_b # [N, C]

Strategy per core (one batch element each). v3: single flat scope so the
Tile list-scheduler can interleave the qkv/v/proj GEMMs as PE "filler"
between attention iterations — the PE<->ACT exp ping-pong otherwise
leaves the PE idle in ~1-5us gaps, which trips the HAM activity
throttle to half clock. Keeping the PE saturated holds 2.4GHz.

  - All matmul operands fp16 (PSUM accumulates fp32; TF32-equivalent
    mantissa, measured rel err ~5e-4 vs 2e-2 budget). Halves input DMA
    and SBUF, and enables fast weight load on the PE.
  - Host pre-transposes x -> xT [C, N], splits qkv_w into wqk (q part
    pre-scaled by d^-0.5) and wv.
  - qkT [2*C, N] fp16 (transposed: per-head qT/kT slices directly feed
    the scores matmuls). v natural [N, C] fp16 augmented with a ones
    column per head (v_aug [N, H*65]) so attn@v also produces softmax
    denominators.
  - Per head pair (2 heads in the 128 partitions): scoresT[k, q] =
    kT-as-lhsT @ qT (K=64 halves), exp on ACT (scores O(1), no max
    subtraction), attn@v accumulated over k chunks into psum [65, N]:
    rows 0-63 = out^T, row 64 = sums. 1/sums via DVE fast reciprocal
    (from an SBUF copy: reading PSUM directly raced the matmul drain on
    HW and produced NaNs), partition-broadcast via DRAM round trip,
    normalize with DVE into fp16 outT.
  - proj: final[t, :] = outT-as-lhsT @ wproj, natural layout, DMA out.
  PSUM budget (8 banks): sc ring 2x[128,1024] = 4 (also carries the
  qkv/v/proj filler groups), av A+B 2x[65,1024] = 4.
"""
import numpy as np

import concourse.bass as bass
import concourse.tile as tile
from concourse import bacc, mybir
from concourse.bass_utils import run_bass_kernel_spmd

DIM = 768
NUM_HEADS = 12
HEAD_DIM = 64
B, N = 8, 1024
P = 128
KC = DIM // P          # 6 contraction chunks of 128 over channels
TC = N // P            # 8 token chunks of 128
QC = N // 512          # 2 free-dim chunks of 512 over tokens
HP = NUM_HEADS // 2    # 6 head pairs
VAUG = 65              # v columns per head: 64 v dims + 1 ones column

F32 = mybir.dt.float32
F32R = mybir.dt.float32r
FP16 = mybir.dt.float16


def build_nc(with_qkv_bias: bool, with_proj_bias: bool):
    nc = bacc.Bacc("TRN2", target_bir_lowering=False, debug=False)

    xT_d = nc.dram_tensor("xT", [DIM, N], FP16, kind="ExternalInput")
    wqk_d = nc.dram_tensor("wqk", [DIM, 2 * DIM], FP16, kind="ExternalInput")
    wv_d = nc.dram_tensor("wv", [DIM, DIM], FP16, kind="ExternalInput")
    wproj_d = nc.dram_tensor("wproj", [DIM, DIM], FP16, kind="ExternalInput")
    bqk_d = nc.dram_tensor("bqk", [1, 2 * DIM], F32, kind="ExternalInput")
    bv_d = nc.dram_tensor("bv", [1, DIM], F32, kind="ExternalInput")
    bproj_d = nc.dram_tensor("bproj", [1, DIM], F32, kind="ExternalInput")
    out_d = nc.dram_tensor("out", [N, DIM], F32, kind="ExternalOutput")

    with tile.TileContext(nc) as tc:
        with (
            tc.tile_pool(name="consts", bufs=1) as consts,
            tc.tile_pool(name="inputs", bufs=1) as in_pool,
            tc.tile_pool(name="qk_sb", bufs=1) as qk_pool,
            tc.tile_pool(name="vaug_sb", bufs=1) as vaug_pool,
            tc.tile_pool(name="outT_sb", bufs=1) as outT_pool,
            tc.tile_pool(name="expT", bufs=6) as exp_pool,
            tc.tile_pool(name="norm", bufs=4) as norm_pool,
            tc.tile_pool(name="rep", bufs=3) as rep_pool,
            tc.tile_pool(name="fin", bufs=3) as fin_pool,
            tc.tile_pool(name="dramp", bufs=1, space="DRAM") as dram_pool,
            tc.tile_pool(name="ps_sc", bufs=2, space="PSUM") as ps_sc,
            tc.tile_pool(name="ps_av", bufs=1, space="PSUM") as ps_av,
        ):
            # ---- ACT exp table preload: a dummy exp so the ~2.7us
            # ACT_TABLE_LOAD happens during the input DMA wait ----
            warm_in = consts.tile([1, 16], F32)
            nc.vector.memset(warm_in[:], 0.0)
            warm_out = consts.tile([1, 16], FP16)
            nc.scalar.activation(
                warm_out[:], warm_in[:], mybir.ActivationFunctionType.Exp
            )

            # ---- constants ----
            if with_qkv_bias or with_proj_bias:
                ones_f32 = consts.tile([1, N], F32)
                nc.vector.memset(ones_f32[:], 1.0)
                ones_row = consts.tile([1, N], FP16)
                nc.vector.tensor_copy(ones_row[:], ones_f32[:])
            if with_qkv_bias:
                bqk_f32 = consts.tile([1, 2 * DIM], F32)
                nc.sync.dma_start(out=bqk_f32[:], in_=bqk_d[:])
                bqk_sb = consts.tile([1, 2 * DIM], FP16)
                nc.vector.tensor_copy(bqk_sb[:], bqk_f32[:])
                bv_f32 = consts.tile([1, DIM], F32)
                nc.sync.dma_start(out=bv_f32[:], in_=bv_d[:])
                bv_sb = consts.tile([1, DIM], FP16)
                nc.vector.tensor_copy(bv_sb[:], bv_f32[:])
            if with_proj_bias:
                bproj_f32 = consts.tile([1, DIM], F32)
                nc.sync.dma_start(out=bproj_f32[:], in_=bproj_d[:])
                bproj_sb = consts.tile([1, DIM], FP16)
                nc.vector.tensor_copy(bproj_sb[:], bproj_f32[:])

            # ---- input DMAs (wqk/xT interleaved so qkT m=0 accumulates
            # as chunks land; wv next, wproj last) ----
            xT = [in_pool.tile([P, N], FP16, name=f"xT{c}") for c in range(KC)]
            wqk = [
                in_pool.tile([P, 2 * DIM], FP16, name=f"wqk{c}") for c in range(KC)
            ]
            wv = [in_pool.tile([P, DIM], FP16, name=f"wv{c}") for c in range(KC)]
            wproj = [
                in_pool.tile([P, DIM], FP16, name=f"wproj{c}") for c in range(KC)
            ]
            for c in range(KC):
                sl = slice(c * P, (c + 1) * P)
                nc.sync.dma_start(out=xT[c][:], in_=xT_d[sl, :])
                nc.sync.dma_start(out=wqk[c][:], in_=wqk_d[sl, :])
            for c in range(KC):
                sl = slice(c * P, (c + 1) * P)
                nc.sync.dma_start(out=wv[c][:], in_=wv_d[sl, :])

            # ---- persistent tiles ----
            qkT = [
                qk_pool.tile([P, N], FP16, name=f"qkT{m}") for m in range(2 * KC)
            ]  # m-chunk m covers qkv channels m*128..m*128+127 (q then k)
            v_aug = [
                vaug_pool.tile([P, NUM_HEADS * VAUG], FP16, name=f"vaug{t}")
                for t in range(TC)
            ]
            outT = [
                outT_pool.tile([P, N], FP16, name=f"outT{p}") for p in range(HP)
            ]  # pair p: head 2p in parts 0-63, head 2p+1 in parts 64-127
            recip_d = dram_pool.tile([NUM_HEADS, N], F32)

            # ---- filler emitters: qkv/v/proj matmul groups the scheduler
            # slots into PE idle gaps during attention. qkT fillers come in
            # half-token-range units to limit sc-ring slot hold time. ----
            def emit_qkT(m, q=None):
                qs = range(QC) if q is None else [q]
                width = N if q is None else 512
                ps = ps_sc.tile([P, width], F32, name=f"ps_qk{m}_{qs[0]}", tag="sc")
                msl = slice(m * P, (m + 1) * P)
                for c in range(KC):
                    for qi, qq in enumerate(qs):
                        qsl = slice(qq * 512, (qq + 1) * 512)
                        osl = slice(qi * 512, (qi + 1) * 512)
                        nc.tensor.matmul(
                            ps[:, osl],
                            wqk[c][:, msl],
                            xT[c][:, qsl],
                            start=(c == 0),
                            stop=(c == KC - 1) and not with_qkv_bias,
                        )
                if with_qkv_bias:
                    for qi, qq in enumerate(qs):
                        qsl = slice(qq * 512, (qq + 1) * 512)
                        osl = slice(qi * 512, (qi + 1) * 512)
                        nc.tensor.matmul(
                            ps[:, osl],
                            bqk_sb[:, msl],
                            ones_row[:, qsl],
                            start=False,
                            stop=True,
                        )
                for qi, qq in enumerate(qs):
                    qsl = slice(qq * 512, (qq + 1) * 512)
                    osl = slice(qi * 512, (qi + 1) * 512)
                    nc.vector.tensor_copy(qkT[m][:, qsl], ps[:, osl])

            def emit_v(t):
                ps = ps_sc.tile([P, DIM], F32, name=f"ps_v{t}", tag="sc")
                tsl = slice(t * P, (t + 1) * P)
                for c in range(KC):
                    for nsl in (slice(0, 512), slice(512, DIM)):
                        nc.tensor.matmul(
                            ps[:, nsl],
                            xT[c][:, tsl],
                            wv[c][:, nsl],
                            start=(c == 0),
                            stop=(c == KC - 1) and not with_qkv_bias,
                        )
                if with_qkv_bias:
                    for nsl in (slice(0, 512), slice(512, DIM)):
                        nc.tensor.matmul(
                            ps[:, nsl],
                            ones_row[:, t * P : t * P + P],
                            bv_sb[:, nsl],
                            start=False,
                            stop=True,
                        )
                va3 = v_aug[t][:].rearrange("p (h e) -> p h e", e=VAUG)
                nc.vector.memset(va3[:, :, 64:65], 1.0)
                nc.vector.tensor_copy(
                    va3[:, :, 0:64],
                    ps[:].rearrange("p (h d) -> p h d", d=HEAD_DIM),
                )

            def emit_proj(t):
                ps = ps_sc.tile([P, DIM], F32, name=f"pj{t}", tag="sc")
                tsl = slice(t * P, (t + 1) * P)
                for c in range(KC):
                    for nsl in (slice(0, 512), slice(512, DIM)):
                        nc.tensor.matmul(
                            ps[:, nsl],
                            outT[c][:, tsl],
                            wproj[c][:, nsl],
                            start=(c == 0),
                            stop=(c == KC - 1) and not with_proj_bias,
                        )
                if with_proj_bias:
                    for nsl in (slice(0, 512), slice(512, DIM)):
                        nc.tensor.matmul(
                            ps[:, nsl],
                            ones_row[:, t * P : t * P + P],
                            bproj_sb[:, nsl],
                            start=False,
                            stop=True,
                        )
                fin = fin_pool.tile([P, DIM], F32, name=f"fin{t}", tag="fin")
                nc.vector.tensor_copy(fin[:], ps[:])
                nc.sync.dma_start(out=out_d[tsl, :], in_=fin[:])

            # Filler schedule: pair 0 consumes v2..v7 at kc 0..5 (v_aug[kc]
            # is needed at AV step kc) and pair 1's qkT halves at kc 6,7;
            # pairs 1..4 consume the next pair's qkT halves at kc 1,3,5,7.
            fillers = [[] for _ in range(HP)]
            fillers[0] = [
                (0, emit_v, (2,)), (1, emit_v, (3,)), (2, emit_v, (4,)),
                (3, emit_v, (5,)), (4, emit_v, (6,)), (5, emit_v, (7,)),
                (6, emit_qkT, (1, 0)), (6, emit_qkT, (1, 1)),
                (7, emit_qkT, (KC + 1, 0)), (7, emit_qkT, (KC + 1, 1)),
            ]
            for p in range(1, HP - 1):
                m = p + 1
                fillers[p] = [
                    (1, emit_qkT, (m, 0)), (3, emit_qkT, (m, 1)),
                    (5, emit_qkT, (KC + m, 0)), (7, emit_qkT, (KC + m, 1)),
                ]

            # ---- prologue: pair 0 operands ----
            emit_qkT(0)
            emit_qkT(KC)
            emit_v(0)
            emit_v(1)

            # ---- attention: 6 head pairs with inline filler ----
            for p in range(HP):
                qT = qkT[p]
                kT = qkT[KC + p]
                av_A = ps_av.tile([VAUG, N], F32, name=f"av{p}A", tag="av")
                av_B = ps_av.tile([VAUG, N], F32, name=f"av{p}B", tag="avB")
                for kc in range(TC):
                    ksl = slice(kc * P, (kc + 1) * P)
                    sc_A = ps_sc.tile([P, N], F32, name=f"sc{p}_{kc}A", tag="sc")
                    sc_B = ps_sc.tile([P, N], F32, name=f"sc{p}_{kc}B", tag="sc")
                    for q in range(QC):
                        qsl = slice(q * 512, (q + 1) * 512)
                        nc.tensor.matmul(
                            sc_A[:, qsl], kT[0:64, ksl], qT[0:64, qsl],
                            start=True, stop=True,
                        )
                        nc.tensor.matmul(
                            sc_B[:, qsl], kT[64:128, ksl], qT[64:128, qsl],
                            start=True, stop=True,
                        )
                    eT_A = exp_pool.tile([P, N], FP16, name=f"e{p}_{kc}A", tag="e")
                    eT_B = exp_pool.tile([P, N], FP16, name=f"e{p}_{kc}B", tag="e")
                    nc.scalar.activation(
                        eT_A[:], sc_A[:], mybir.ActivationFunctionType.Exp
                    )
                    nc.scalar.activation(
                        eT_B[:], sc_B[:], mybir.ActivationFunctionType.Exp
                    )
                    for q in range(QC):
                        qsl = slice(q * 512, (q + 1) * 512)
                        nc.tensor.matmul(
                            av_A[:, qsl],
                            v_aug[kc][:, 2 * p * VAUG : (2 * p + 1) * VAUG],
                            eT_A[:, qsl],
                            start=(kc == 0), stop=(kc == TC - 1),
                        )
                        nc.tensor.matmul(
                            av_B[:, qsl],
                            v_aug[kc][:, (2 * p + 1) * VAUG : (2 * p + 2) * VAUG],
                            eT_B[:, qsl],
                            start=(kc == 0), stop=(kc == TC - 1),
                        )
                    for fkc, fn, args in fillers[p]:
                        if fkc == kc:
                            fn(*args)
                if p == HP - 2:
                    # wproj can land any time before proj; issue during pair 4
                    for c in range(KC):
                        nc.sync.dma_start(
                            out=wproj[c][:],
                            in_=wproj_d[c * P : (c + 1) * P, :],
                        )
                # ---- normalization: sums -> 1/sums -> broadcast across
                # partitions via DRAM round trip -> DVE multiply ----
                for hh, av in ((2 * p, av_A), (2 * p + 1, av_B)):
                    sums_t = norm_pool.tile([1, N], F32, name=f"sums{hh}", tag="sums")
                    recip_t = norm_pool.tile(
                        [1, N], F32, name=f"recip{hh}", tag="recip"
                    )
                    nc.vector.tensor_copy(sums_t[:], av[64:65, :])
                    nc.vector.reciprocal_approx_fast(
                        out=recip_t[:], in_=sums_t[:]
                    )
                    nc.sync.dma_start(out=recip_d[hh : hh + 1, :], in_=recip_t[:])
                for hh, av in ((2 * p, av_A), (2 * p + 1, av_B)):
                    rep = rep_pool.tile([64, N], F32, name=f"rep{hh}", tag="rep")
                    nc.sync.dma_start(
                        out=rep[:],
                        in_=recip_d[hh : hh + 1, :].to_broadcast([64, N]),
                    )
                    half = slice(0, 64) if hh % 2 == 0 else slice(64, 128)
                    nc.vector.tensor_tensor(
                        out=outT[p][half, :],
                        in0=av[0:64, :],
                        in1=rep[:],
                        op=mybir.AluOpType.mult,
                    )

            # ---- proj: sc ring is free once attention is done; c=0..4
            # accumulation can overlap pair 5's tail ----
            for t in range(TC):
                emit_proj(t)

    nc.compile()
    return nc


_NC_CACHE = {}


def kernel(**inputs) -> np.ndarray:
    x = np.asarray(inputs["x"], dtype=np.float32)
    qkv_w = np.asarray(inputs["qkv_w"], dtype=np.float32)
    qkv_b = np.asarray(inputs["qkv_b"], dtype=np.float32)
    proj_w = np.asarray(inputs["proj_w"], dtype=np.float32)
    proj_b = np.asarray(inputs["proj_b"], dtype=np.float32)
    # context is unused by the reference layer.

    scale = HEAD_DIM ** -0.5
    wqk = qkv_w[:, : 2 * DIM].copy()
    wqk[:, :DIM] *= scale
    wv = np.ascontiguousarray(qkv_w[:, 2 * DIM :])
    bqk = qkv_b[: 2 * DIM].copy()
    bqk[:DIM] *= scale
    bv = qkv_b[2 * DIM :].copy()

    with_qkv_bias = bool(np.any(qkv_b))
    with_proj_bias = bool(np.any(proj_b))

    key = (with_qkv_bias, with_proj_bias)
    if key not in _NC_CACHE:
        _NC_CACHE[key] = build_nc(*key)
    nc = _NC_CACHE[key]

    base = {
        "wqk": wqk.astype(np.float16),
        "wv": wv.astype(np.float16),
        "wproj": proj_w.astype(np.float16),
        "bqk": bqk.reshape(1, -1),
        "bv": bv.reshape(1, -1),
        "bproj": proj_b.reshape(1, -1),
    }
    in_maps = [
        {**base, "xT": np.ascontiguousarray(x[b].T).astype(np.float16)}
        for b in range(B)
    ]
    res = run_bass_kernel_spmd(nc, in_maps, list(range(B)))
    out = np.stack([res.results[b]["out"] for b in range(B)], axis=0)
    return out.astype(np.float32)
